# revision 18
# baseline (speedup 1.0000x reference)
"""Bidirectional LSTM + vocab projection for 8 Trainium2 NeuronCores.

Sharding: data-parallel over batch — core c owns batch elements {2c, 2c+1}
(B=16, 8 cores). Each core runs the full recurrence for its 2 batches and
the full-vocab FC for its 512 token columns. No collectives; per-core
inputs differ only in the gather indices. Host reassembles [B, T, V].

Per-core program:
  - Embedding gather (indirect DMA) for local tokens in natural (col=t*2+b)
    and time-reversed order; PE transpose to x^T (E on partitions), bf16.
  - Recurrence in "orientation B": gate GEMM computes g^T [gates, batch]
    with the weight tile as the stationary operand and the 2-wide batch as
    the moving operand, so each matmul streams only 2 rows. Gate columns of
    both dirs for 8 steps accumulate into one PSUM window; the x@Wx part is
    issued once per 8-step window with a strided 3D out-AP.
  - Cell update uses sigma(x) = (1+tanh(x/2))/2: one tanh(0.5*g) over all
    gates, then fused scalar_tensor_tensor ops; cell/h are stored doubled
    (C=2c, H=2h) with the compensating 0.5/2 factors baked into the weights
    host-side, which keeps the whole update at 1 ACT + 3 STT + 1 ACT + 2 STT.
  - h^T (bf16, doubled) lands directly in its resident SBUF store in token
    order for both directions (no per-step transposes).
  - FC: out^T[vocab_tile, 512 tok] = (0.5*fc_w)^T @ H_cat per 128-vocab
    tile; fc_w streamed from DRAM through a prefetch ring; bf16 output.
"""

import numpy as np
from contextlib import ExitStack

import ml_dtypes
import concourse.bass as bass
import concourse.tile as tile
from concourse import bacc, mybir
from concourse.bass_utils import run_bass_kernel_spmd

N_CORES = 8
B, T, V, E, H = 16, 256, 32000, 256, 512
BL = B // N_CORES                 # 2 local batches
NTOK = BL * T                     # 512 local token cols, col = t*2+b
VT = V // 128                     # 250 vocab tiles
KR = 6                            # K chunks: 2x x (E) + 4x h (H)
WIN = 8                           # recurrence steps per PSUM window
NPRE = 0                          # vocab tiles whose middle cols overlap rec
RING = 8                          # fcw prefetch ring depth

f32 = mybir.dt.float32
bf16 = mybir.dt.bfloat16
i32 = mybir.dt.int32

_CACHE = {}

# gate order on device: i, f, chat, o  <-  reference order i, f, o, chat
GMAP = [0, 1, 3, 2]


def _build(t_steps=None, debug_dump=False):
    if t_steps is None:
        t_steps = T
    assert t_steps % WIN == 0
    nc = bacc.Bacc("TRN2", target_bir_lowering=False, debug=False,
                   num_devices=N_CORES)

    idx_d = nc.dram_tensor("idx", [8, 128], i32, kind="ExternalInput").ap()
    emb_d = nc.dram_tensor("emb", [V, E], f32, kind="ExternalInput").ap()
    wcat_d = nc.dram_tensor("wcat", [2, KR, 128, 16 * 128], bf16,
                            kind="ExternalInput").ap()
    fcw_d = nc.dram_tensor("fcw", [VT, 128, 8 * 128], bf16,
                           kind="ExternalInput").ap()
    ident_d = nc.dram_tensor("ident", [128, 128], f32, kind="ExternalInput").ap()
    out_d = nc.dram_tensor("logits", [VT, 128, NTOK], bf16,
                           kind="ExternalOutput").ap()
    if debug_dump:
        dbg_g = nc.dram_tensor("dbg_g", [t_steps, 128, 64], f32,
                               kind="ExternalOutput").ap()
        dbg_hT = nc.dram_tensor("dbg_hT", [128, 8 * NTOK], f32,
                                kind="ExternalOutput").ap()

    with tile.TileContext(nc) as tc, ExitStack() as top:
        const_pool = top.enter_context(tc.tile_pool(name="const", bufs=1))
        ident_sb = const_pool.tile([128, 128], f32)
        nc.sync.dma_start(ident_sb[:], ident_d[:])
        idx_sb = const_pool.tile([128, 8], i32)
        nc.sync.dma_start(idx_sb[:], idx_d.rearrange("a b -> b a"))

        # recurrence weights: per (dir, k-chunk) one tile, m-tiles side by side
        wp = top.enter_context(tc.tile_pool(name="wp", bufs=1))
        w_sb = [[None] * KR for _ in range(2)]
        for d in range(2):
            for k in range(KR):
                w_sb[d][k] = wp.tile([128, 16 * 128], bf16, name=f"w{d}_{k}")
                nc.sync.dma_start(w_sb[d][k][:], wcat_d[d, k])

        # H^T resident store: chunk kk = 4*dir + unit_group, col = t*2 + b
        hT_pool = top.enter_context(tc.tile_pool(name="hTp", bufs=1))
        hT_all = hT_pool.tile([128, 8 * NTOK], bf16)

        xt_pool = top.enter_context(tc.tile_pool(name="xt", bufs=1))
        # xT[d][hf]: E-half hf on partitions; d=0 natural, d=1 time-reversed
        xT = [[xt_pool.tile([128, NTOK], bf16, name=f"xT{d}_{hf}")
               for hf in range(2)] for d in range(2)]

        # ---- gather + transpose x^T ----
        with ExitStack() as gctx:
            gat_pool = gctx.enter_context(tc.tile_pool(name="gat", bufs=4))
            gps_pool = gctx.enter_context(
                tc.tile_pool(name="gps", bufs=4, space="PSUM"))
            for i in range(8):
                d, it = i // 4, i % 4
                x_nat = gat_pool.tile([128, E], f32, tag="xnat")
                nc.gpsimd.indirect_dma_start(
                    out=x_nat[:], out_offset=None, in_=emb_d[:],
                    in_offset=bass.IndirectOffsetOnAxis(
                        ap=idx_sb[:, i:i + 1], axis=0))
                for hf in range(2):
                    xp = gps_pool.tile([128, 128], f32, tag="xp")
                    nc.tensor.transpose(
                        xp[:], x_nat[:, 128 * hf:128 * (hf + 1)], ident_sb[:])
                    nc.vector.tensor_copy(
                        xT[d][hf][:, 128 * it:128 * (it + 1)], xp[:])

        # ---- recurrence state ----
        st_pool = top.enter_context(tc.tile_pool(name="st", bufs=1))
        hT_zero = st_pool.tile([128, 2], bf16)
        nc.vector.memset(hT_zero[:], 0.0)
        c_sb = st_pool.tile([128, 16], f32)   # col = d*8 + q*2 + b
        nc.vector.memset(c_sb[:], 0.0)

        ps_pool = top.enter_context(
            tc.tile_pool(name="rps", bufs=1, space="PSUM"))
        gw_bufs = [ps_pool.tile([128, WIN * 64], f32, name=f"gwb{i}")
                   for i in range(2)]
        ew_pool = top.enter_context(tc.tile_pool(name="ew", bufs=3))

        # FC prefetch ring + psum (declared up front so FC can interleave
        # with the recurrence). Phase R (steps >= T-NPRE/2*... i.e. once the
        # middle token block is available) computes cols [128,384) for the
        # first NPRE vocab tiles; phase E finishes their edge columns plus
        # the remaining tiles in full.
        fcw_pool = top.enter_context(tc.tile_pool(name="fcw", bufs=1))
        fpsR_pool = top.enter_context(
            tc.tile_pool(name="fpsR", bufs=1, space="PSUM"))
        fpsE_pool = top.enter_context(
            tc.tile_pool(name="fpsE", bufs=1, space="PSUM"))
        ev_pool = top.enter_context(tc.tile_pool(name="ev", bufs=1))
        evb_pool = top.enter_context(tc.tile_pool(name="evb", bufs=1))
        ring = [None] * RING

        def fc_stream(v):
            t = fcw_pool.tile([128, 8 * 128], bf16, tag=f"fcw{v % RING}")
            nc.sync.dma_start(t[:], fcw_d[v])
            ring[v % RING] = t

        def fc_mid(v):
            # cols [128, 384) for vocab tile v; batched output DMA per 8
            fcw_t = ring[v % RING]
            pf = fpsR_pool.tile([128, 256], f32, tag=f"pfr{v % 2}")
            for k in range(8):
                nc.tensor.matmul(
                    pf[:], fcw_t[:, 128 * k:128 * (k + 1)],
                    hT_all[:, NTOK * k + 128:NTOK * k + 384],
                    start=(k == 0), stop=(k == 7))
            evb = evb_pool.tile([128, 8 * 256], bf16, tag=f"evb{(v // 8) % 2}")
            if v % 2 == 0:
                nc.vector.tensor_copy(
                    evb[:, 256 * (v % 8):256 * (v % 8) + 256], pf[:])
            else:
                nc.scalar.copy(
                    evb[:, 256 * (v % 8):256 * (v % 8) + 256], pf[:])
            if v % 8 == 7:
                nc.sync.dma_start(
                    out_d[v - 7:v + 1, :, 128:384],
                    evb[:].rearrange("p (v n) -> v p n", v=8))

        def fc_edges(v):
            # cols [0,128) and [384,512) for vocab tile v (phase E)
            fcw_t = ring[v % RING]
            pf = fpsE_pool.tile([128, 256], f32, tag=f"pfe{v % 2}")
            for e, c0 in enumerate((0, 384)):
                for k in range(8):
                    nc.tensor.matmul(
                        pf[:, 128 * e:128 * (e + 1)],
                        fcw_t[:, 128 * k:128 * (k + 1)],
                        hT_all[:, NTOK * k + c0:NTOK * k + c0 + 128],
                        start=(k == 0), stop=(k == 7))
            evA = evb_pool.tile([128, 8 * 128], bf16, tag=f"evA{(v // 8) % 2}")
            evB = evb_pool.tile([128, 8 * 128], bf16, tag=f"evB{(v // 8) % 2}")
            if v % 2 == 0:
                nc.vector.tensor_copy(
                    evA[:, 128 * (v % 8):128 * (v % 8) + 128], pf[:, 0:128])
                nc.scalar.copy(
                    evB[:, 128 * (v % 8):128 * (v % 8) + 128], pf[:, 128:256])
            else:
                nc.scalar.copy(
                    evA[:, 128 * (v % 8):128 * (v % 8) + 128], pf[:, 0:128])
                nc.vector.tensor_copy(
                    evB[:, 128 * (v % 8):128 * (v % 8) + 128], pf[:, 128:256])
            if v % 8 == 7:
                nc.sync.dma_start(out_d[v - 7:v + 1, :, 0:128],
                                  evA[:].rearrange("p (v n) -> v p n", v=8))
                nc.sync.dma_start(out_d[v - 7:v + 1, :, 384:512],
                                  evB[:].rearrange("p (v n) -> v p n", v=8))

        def fc_full(v):
            fcw_t = ring[v % RING]
            pf = fpsE_pool.tile([128, NTOK], f32, tag=f"pff{v % 2}")
            for k in range(8):
                nc.tensor.matmul(
                    pf[:], fcw_t[:, 128 * k:128 * (k + 1)],
                    hT_all[:, NTOK * k:NTOK * (k + 1)],
                    start=(k == 0), stop=(k == 7))
            ev = ev_pool.tile([128, NTOK], bf16, tag=f"ev{v % 3}")
            if v % 2 == 0:
                nc.vector.tensor_copy(ev[:], pf[:])
            else:
                nc.scalar.copy(ev[:], pf[:])
            nc.sync.dma_start(out_d[v], ev[:])

        # PSUM column layout per step (gate-major): col = g*16 + d*8 + q*2+b
        # with g in {i:0, f:1, chat:2, o:3}. Every elementwise operand is a
        # flat contiguous slice, and the i/f/chat block [0,48) excludes the
        # o matmuls from the ACT dependency.
        def gcol(d, m):
            return (m // 4) * 16 + 8 * d + 2 * (m % 4)

        # Accumulate with start=False everywhere onto pre-zeroed banks: a
        # start=True (first_mm) clears has_written for the WHOLE bank, which
        # breaks interleaved accumulation groups; accumulating onto zeroed
        # values is order-independent and hw-bit-agnostic. Both window
        # buffers are zeroed upfront; each step re-zeroes its block of the
        # next window right after its H write (GPSIMD cannot touch PSUM).
        nc.vector.memset(gw_bufs[0][:], 0.0)
        nc.vector.memset(gw_bufs[1][:], 0.0)
        n_win = t_steps // WIN
        for w in range(n_win):
            gw = gw_bufs[w % 2]
            gw3 = gw[:].rearrange("p (si c) -> p si c", si=WIN)
            # x part for the whole window: out [128, (8 si, 2 b)]
            for d in range(2):
                for k in range(2):
                    rhs = xT[d][k][:, w * 2 * WIN:(w + 1) * 2 * WIN] \
                        .rearrange("p (si b) -> p si b", si=WIN)
                    for m in range(16):
                        c0 = gcol(d, m)
                        nc.tensor.matmul(
                            gw3[:, :, c0:c0 + 2],
                            w_sb[d][k][:, 128 * m:128 * (m + 1)],
                            rhs, start=False, stop=False,
                            skip_group_check=True)
            for si in range(WIN):
                s = WIN * w + si
                # h part: i,f,chat tiles first; o tiles off the critical path
                for m0, m1 in ((0, 12), (12, 16)):
                    for d in range(2):
                        tok_prev = s - 1 if d == 0 else T - s
                        for m in range(m0, m1):
                            c0 = 64 * si + gcol(d, m)
                            for k in range(2, KR):
                                q = k - 2
                                if s == 0:
                                    rhs = hT_zero[:]
                                else:
                                    o = (4 * d + q) * NTOK + 2 * tok_prev
                                    rhs = hT_all[:, o:o + 2]
                                nc.tensor.matmul(
                                    gw[:, c0:c0 + 2],
                                    w_sb[d][k][:, 128 * m:128 * (m + 1)],
                                    rhs, start=False, stop=(k == KR - 1),
                                    skip_group_check=True)

                # ---- cell update ----
                gv = gw[:, 64 * si:64 * si + 64]
                if debug_dump:
                    dgt = ew_pool.tile([128, 64], f32, tag="dbgg")
                    nc.scalar.copy(dgt[:], gv)
                    nc.sync.dma_start(dbg_g[s], dgt[:])
                t_t = ew_pool.tile([128, 64], f32, tag="tt")
                # t = tanh(0.5*g): one op for i,f,chat; o separately (it is
                # only needed at the very end of the chain)
                nc.scalar.activation(t_t[:, 0:48], gv[:, 0:48],
                                     mybir.ActivationFunctionType.Tanh,
                                     scale=0.5)
                nc.scalar.activation(t_t[:, 48:64], gv[:, 48:64],
                                     mybir.ActivationFunctionType.Tanh,
                                     scale=0.5)
                qt = ew_pool.tile([128, 16], f32, tag="qt")
                nc.vector.scalar_tensor_tensor(
                    qt[:], t_t[:, 16:32], 1.0, c_sb[:],
                    op0=mybir.AluOpType.add, op1=mybir.AluOpType.mult)
                pt = ew_pool.tile([128, 16], f32, tag="pt")
                nc.vector.scalar_tensor_tensor(
                    pt[:], t_t[:, 0:16], 1.0, t_t[:, 32:48],
                    op0=mybir.AluOpType.add, op1=mybir.AluOpType.mult)
                # C' = 0.5*q + p  (C = 2c)
                nc.vector.scalar_tensor_tensor(
                    c_sb[:], qt[:], 0.5, pt[:],
                    op0=mybir.AluOpType.mult, op1=mybir.AluOpType.add)
                tanc = ew_pool.tile([128, 16], f32, tag="tanc")
                nc.scalar.activation(tanc[:], c_sb[:],
                                     mybir.ActivationFunctionType.Tanh,
                                     scale=0.5)
                # H = (t_o + 1) * tanh(c); straight into hT_all, bf16
                hT8 = hT_all[:].rearrange("p (kk n) -> p kk n", kk=8)
                for d in range(2):
                    tok = s if d == 0 else T - 1 - s
                    nc.vector.scalar_tensor_tensor(
                        hT8[:, 4 * d:4 * d + 4, 2 * tok:2 * tok + 2],
                        t_t[:, 48 + 8 * d:48 + 8 * d + 8]
                        .rearrange("p (q b) -> p q b", q=4),
                        1.0,
                        tanc[:, 8 * d:8 * d + 8]
                        .rearrange("p (q b) -> p q b", q=4),
                        op0=mybir.AluOpType.add, op1=mybir.AluOpType.mult)
                if w + 1 < n_win:
                    nc.vector.memset(
                        gw_bufs[(w + 1) % 2][:, 64 * si:64 * si + 64], 0.0)

                # FC interleave: middle cols of 2 vocab tiles per step once
                # tokens [64,192) are complete (s >= 192); fcw prefetched a
                # few steps ahead so FC matmuls never stall the PE queue.
                if t_steps == T:
                    sp = s + 3                       # prefetch lead
                    if T - NPRE // 2 <= sp < T:
                        for v in (2 * (sp - (T - NPRE // 2)),
                                  2 * (sp - (T - NPRE // 2)) + 1):
                            fc_stream(v)
                    if s >= T - NPRE // 2:
                        for v in (2 * (s - (T - NPRE // 2)),
                                  2 * (s - (T - NPRE // 2)) + 1):
                            fc_mid(v)

        if debug_dump:
            dh = ew_pool.tile([128, 8 * NTOK], f32, tag="dbgh")
            nc.vector.tensor_copy(dh[:], hT_all[:])
            nc.sync.dma_start(dbg_hT[:], dh[:])

        # ---- FC phase E: remaining full tiles, then edge cols of the
        # phase-R tiles (their fcw is re-streamed) ----
        if t_steps == T:
            order = list(range(NPRE, VT)) + list(range(NPRE))
            depth = RING - 2
            for v in order[:depth]:
                fc_stream(v)
            for i, v in enumerate(order):
                if i + depth < len(order):
                    fc_stream(order[i + depth])
                if v >= NPRE:
                    fc_full(v)
                else:
                    fc_edges(v)
        else:
            for v in range(VT):
                fc_stream(v)
                fc_full(v)

    nc.compile()
    return nc


def _host_prep(inputs, emb, Wh_fwd, Wx_fwd, b_fwd, Wh_bwd, Wx_bwd, b_bwd,
               fc_w, fc_b):
    inp = np.asarray(inputs).astype(np.int32)          # [B, T]
    emb = np.ascontiguousarray(np.asarray(emb, dtype=np.float32))

    wcat = np.zeros((2, KR, 128, 16 * 128), dtype=np.float32)
    for d, (Wh, Wx) in enumerate(((Wh_fwd, Wx_fwd), (Wh_bwd, Wx_bwd))):
        Wh = np.asarray(Wh, dtype=np.float32)
        Wx = np.asarray(Wx, dtype=np.float32)
        Wfull = np.zeros((E + H, 4 * H), dtype=np.float32)
        for gm in range(4):
            gr = GMAP[gm]
            Wfull[:E, gm * H:(gm + 1) * H] = Wx[gr]
            Wfull[E:, gm * H:(gm + 1) * H] = Wh[gr] * 0.5
        Wfull[:, 2 * H:3 * H] *= 2.0                   # chat columns
        wcat[d] = Wfull.reshape(KR, 128, 16 * 128)
    wcat = wcat.astype(ml_dtypes.bfloat16)

    fc_w = np.asarray(fc_w, dtype=np.float32) * 0.5    # H = 2h
    fcw = np.ascontiguousarray(
        fc_w.reshape(8, 128, VT, 128).transpose(2, 1, 0, 3)
        .reshape(VT, 128, 8 * 128)).astype(ml_dtypes.bfloat16)

    ident = np.eye(128, dtype=np.float32)
    ts = np.arange(T)
    in_maps = []
    for c in range(N_CORES):
        idx = np.zeros((8, 128), dtype=np.int32)
        for b in range(BL):
            loc = inp[BL * c + b]                      # [T]
            idx.reshape(2, 4 * 128)[0, 2 * ts + b] = loc
            idx.reshape(2, 4 * 128)[1, 2 * ts + b] = loc[::-1]
        in_maps.append(dict(idx=idx, emb=emb, wcat=wcat, fcw=fcw,
                            ident=ident))
    lstm_bias_zero = (not np.any(np.asarray(b_fwd))) and \
        (not np.any(np.asarray(b_bwd)))
    return in_maps, lstm_bias_zero


def run(in_maps, nc=None, **build_kw):
    if nc is None:
        key = tuple(sorted(build_kw.items()))
        if key not in _CACHE:
            _CACHE[key] = _build(**build_kw)
        nc = _CACHE[key]
    res = run_bass_kernel_spmd(nc, in_maps, core_ids=list(range(N_CORES)))
    return res


def kernel(**inputs):
    in_maps, lstm_bias_zero = _host_prep(**inputs)
    assert lstm_bias_zero, "nonzero LSTM biases not supported by this build"
    res = run(in_maps)
    ts = np.arange(T)
    out = np.empty((B, T, V), dtype=np.float32)
    for c in range(N_CORES):
        lg = np.asarray(res.results[c]["logits"]).reshape(V, NTOK)
        lg = lg.astype(np.float32)
        for b in range(BL):
            out[BL * c + b] = lg[:, 2 * ts + b].T
    fc_b = np.asarray(inputs["fc_b"], dtype=np.float32)
    if np.any(fc_b):
        out += fc_b
    return out


# revision 23
# speedup vs baseline: 1.0776x; 1.0776x over previous
"""Bidirectional LSTM + vocab projection for 8 Trainium2 NeuronCores.

Sharding: data-parallel over batch — core c owns batch elements {2c, 2c+1}
(B=16, 8 cores). Each core runs the full recurrence for its 2 batches and
the full-vocab FC for its 512 token columns. No collectives; per-core
inputs differ only in the gather indices. Host reassembles [B, T, V].

Per-core program:
  - Embedding gather (indirect DMA) for local tokens in natural (col=t*2+b)
    and time-reversed order; PE transpose to x^T (E on partitions), bf16.
  - Recurrence in "orientation B": gate GEMM computes g^T [gates, batch]
    with the weight tile as the stationary operand and the 2-wide batch as
    the moving operand, so each matmul streams only 2 rows. Gate columns of
    both dirs for 8 steps accumulate into one PSUM window; the x@Wx part is
    issued once per 8-step window with a strided 3D out-AP.
  - Cell update uses sigma(x) = (1+tanh(x/2))/2: one tanh(0.5*g) over all
    gates, then fused scalar_tensor_tensor ops; cell/h are stored doubled
    (C=2c, H=2h) with the compensating 0.5/2 factors baked into the weights
    host-side, which keeps the whole update at 1 ACT + 3 STT + 1 ACT + 2 STT.
  - h^T (bf16, doubled) lands directly in its resident SBUF store in token
    order for both directions (no per-step transposes).
  - FC: out^T[vocab_tile, 512 tok] = (0.5*fc_w)^T @ H_cat per 128-vocab
    tile; fc_w streamed from DRAM through a prefetch ring; bf16 output.
"""

import numpy as np
from contextlib import ExitStack

import ml_dtypes
import concourse.bass as bass
import concourse.tile as tile
from concourse import bacc, mybir
from concourse.bass_utils import run_bass_kernel_spmd

N_CORES = 8
B, T, V, E, H = 16, 256, 32000, 256, 512
BL = B // N_CORES                 # 2 local batches
NTOK = BL * T                     # 512 local token cols, col = t*2+b
VT = V // 128                     # 250 vocab tiles
KR = 6                            # K chunks: 2x x (E) + 4x h (H)
WIN = 8                           # recurrence steps per PSUM window
NPRE = 128                        # vocab tiles whose middle cols overlap rec
RING = 8                          # fcw prefetch ring depth

f32 = mybir.dt.float32
bf16 = mybir.dt.bfloat16
i32 = mybir.dt.int32

_CACHE = {}

# gate order on device: i, f, chat, o  <-  reference order i, f, o, chat
GMAP = [0, 1, 3, 2]


def _build(t_steps=None, debug_dump=False):
    if t_steps is None:
        t_steps = T
    assert t_steps % WIN == 0
    nc = bacc.Bacc("TRN2", target_bir_lowering=False, debug=False,
                   num_devices=N_CORES)

    idx_d = nc.dram_tensor("idx", [8, 128], i32, kind="ExternalInput").ap()
    emb_d = nc.dram_tensor("emb", [V, E], f32, kind="ExternalInput").ap()
    wcat_d = nc.dram_tensor("wcat", [2, KR, 128, 16 * 128], bf16,
                            kind="ExternalInput").ap()
    fcw_d = nc.dram_tensor("fcw", [VT, 128, 8 * 128], bf16,
                           kind="ExternalInput").ap()
    ident_d = nc.dram_tensor("ident", [128, 128], f32, kind="ExternalInput").ap()
    out_d = nc.dram_tensor("logits", [VT, 128, NTOK], bf16,
                           kind="ExternalOutput").ap()
    if debug_dump:
        dbg_g = nc.dram_tensor("dbg_g", [t_steps, 128, 64], f32,
                               kind="ExternalOutput").ap()
        dbg_hT = nc.dram_tensor("dbg_hT", [128, 8 * NTOK], f32,
                                kind="ExternalOutput").ap()

    with tile.TileContext(nc) as tc, ExitStack() as top:
        const_pool = top.enter_context(tc.tile_pool(name="const", bufs=1))
        ident_sb = const_pool.tile([128, 128], f32)
        nc.sync.dma_start(ident_sb[:], ident_d[:])
        idx_sb = const_pool.tile([128, 8], i32)
        nc.sync.dma_start(idx_sb[:], idx_d.rearrange("a b -> b a"))

        # recurrence weights: per (dir, k-chunk) one tile, m-tiles side by side
        wp = top.enter_context(tc.tile_pool(name="wp", bufs=1))
        w_sb = [[None] * KR for _ in range(2)]
        for d in range(2):
            for k in range(KR):
                w_sb[d][k] = wp.tile([128, 16 * 128], bf16, name=f"w{d}_{k}")
                nc.sync.dma_start(w_sb[d][k][:], wcat_d[d, k])

        # H^T resident store: chunk kk = 4*dir + unit_group, col = t*2 + b
        hT_pool = top.enter_context(tc.tile_pool(name="hTp", bufs=1))
        hT_all = hT_pool.tile([128, 8 * NTOK], bf16)

        xt_pool = top.enter_context(tc.tile_pool(name="xt", bufs=1))
        # xT[d][hf]: E-half hf on partitions; d=0 natural, d=1 time-reversed
        xT = [[xt_pool.tile([128, NTOK], bf16, name=f"xT{d}_{hf}")
               for hf in range(2)] for d in range(2)]

        # ---- gather + transpose x^T ----
        with ExitStack() as gctx:
            gat_pool = gctx.enter_context(tc.tile_pool(name="gat", bufs=4))
            gps_pool = gctx.enter_context(
                tc.tile_pool(name="gps", bufs=4, space="PSUM"))
            for i in range(8):
                d, it = i // 4, i % 4
                x_nat = gat_pool.tile([128, E], f32, tag="xnat")
                nc.gpsimd.indirect_dma_start(
                    out=x_nat[:], out_offset=None, in_=emb_d[:],
                    in_offset=bass.IndirectOffsetOnAxis(
                        ap=idx_sb[:, i:i + 1], axis=0))
                for hf in range(2):
                    xp = gps_pool.tile([128, 128], f32, tag="xp")
                    nc.tensor.transpose(
                        xp[:], x_nat[:, 128 * hf:128 * (hf + 1)], ident_sb[:])
                    nc.vector.tensor_copy(
                        xT[d][hf][:, 128 * it:128 * (it + 1)], xp[:])

        # ---- recurrence state ----
        st_pool = top.enter_context(tc.tile_pool(name="st", bufs=1))
        hT_zero = st_pool.tile([128, 2], bf16)
        nc.vector.memset(hT_zero[:], 0.0)
        c_sb = st_pool.tile([128, 16], f32)   # col = d*8 + q*2 + b
        nc.vector.memset(c_sb[:], 0.0)

        ps_pool = top.enter_context(
            tc.tile_pool(name="rps", bufs=1, space="PSUM"))
        gw_bufs = [ps_pool.tile([128, WIN * 64], f32, name=f"gwb{i}")
                   for i in range(2)]
        ew_pool = top.enter_context(tc.tile_pool(name="ew", bufs=3))

        # FC prefetch ring + psum (declared up front so FC can interleave
        # with the recurrence). Phase R (steps >= T-NPRE/2*... i.e. once the
        # middle token block is available) computes cols [128,384) for the
        # first NPRE vocab tiles; phase E finishes their edge columns plus
        # the remaining tiles in full.
        fcw_pool = top.enter_context(tc.tile_pool(name="fcw", bufs=1))
        fpsR_pool = top.enter_context(
            tc.tile_pool(name="fpsR", bufs=1, space="PSUM"))
        fpsE_pool = top.enter_context(
            tc.tile_pool(name="fpsE", bufs=1, space="PSUM"))
        ev_pool = top.enter_context(tc.tile_pool(name="ev", bufs=1))
        evb_pool = top.enter_context(tc.tile_pool(name="evb", bufs=1))
        # Two independent rings: "A" for phase-R + full tiles, "E" for the
        # phase-E edge re-streams. Separate tags keep the slot-rotation WAR
        # chains linear (a same-tag stream must never be emitted before the
        # previous tile with that tag has its readers emitted).
        rings = {"A": [None] * RING, "E": [None] * RING}

        def fc_stream(v, rk="A"):
            t = fcw_pool.tile([128, 8 * 128], bf16, tag=f"fcw{rk}{v % RING}")
            nc.sync.dma_start(t[:], fcw_d[v])
            rings[rk][v % RING] = t

        def fc_mid(v):
            # cols [128, 384) for vocab tile v; batched output DMA per 8
            fcw_t = rings["A"][v % RING]
            pf = fpsR_pool.tile([128, 256], f32, tag=f"pfr{v % 2}")
            for k in range(8):
                nc.tensor.matmul(
                    pf[:], fcw_t[:, 128 * k:128 * (k + 1)],
                    hT_all[:, NTOK * k + 128:NTOK * k + 384],
                    start=(k == 0), stop=(k == 7))
            evb = evb_pool.tile([128, 8 * 256], bf16, tag=f"evb{(v // 8) % 2}")
            if v % 2 == 0:
                nc.vector.tensor_copy(
                    evb[:, 256 * (v % 8):256 * (v % 8) + 256], pf[:])
            else:
                nc.scalar.copy(
                    evb[:, 256 * (v % 8):256 * (v % 8) + 256], pf[:])
            if v % 8 == 7:
                nc.sync.dma_start(
                    out_d[v - 7:v + 1, :, 128:384]
                    .rearrange("v p n -> p v n"),
                    evb[:].rearrange("p (v n) -> p v n", v=8))

        def fc_edges(v):
            # cols [0,128) and [384,512) for vocab tile v (phase E)
            fcw_t = rings["E"][v % RING]
            pf = fpsE_pool.tile([128, 256], f32, tag=f"pfe{v % 2}")
            for e, c0 in enumerate((0, 384)):
                for k in range(8):
                    nc.tensor.matmul(
                        pf[:, 128 * e:128 * (e + 1)],
                        fcw_t[:, 128 * k:128 * (k + 1)],
                        hT_all[:, NTOK * k + c0:NTOK * k + c0 + 128],
                        start=(k == 0), stop=(k == 7))
            evA = evb_pool.tile([128, 8 * 128], bf16, tag=f"evA{(v // 8) % 2}")
            evB = evb_pool.tile([128, 8 * 128], bf16, tag=f"evB{(v // 8) % 2}")
            if v % 2 == 0:
                nc.vector.tensor_copy(
                    evA[:, 128 * (v % 8):128 * (v % 8) + 128], pf[:, 0:128])
                nc.scalar.copy(
                    evB[:, 128 * (v % 8):128 * (v % 8) + 128], pf[:, 128:256])
            else:
                nc.scalar.copy(
                    evA[:, 128 * (v % 8):128 * (v % 8) + 128], pf[:, 0:128])
                nc.vector.tensor_copy(
                    evB[:, 128 * (v % 8):128 * (v % 8) + 128], pf[:, 128:256])
            if v % 8 == 7:
                nc.sync.dma_start(out_d[v - 7:v + 1, :, 0:128]
                                  .rearrange("v p n -> p v n"),
                                  evA[:].rearrange("p (v n) -> p v n", v=8))
                nc.sync.dma_start(out_d[v - 7:v + 1, :, 384:512]
                                  .rearrange("v p n -> p v n"),
                                  evB[:].rearrange("p (v n) -> p v n", v=8))

        def fc_full(v):
            fcw_t = rings["A"][v % RING]
            pf = fpsE_pool.tile([128, NTOK], f32, tag=f"pff{v % 2}")
            for k in range(8):
                nc.tensor.matmul(
                    pf[:], fcw_t[:, 128 * k:128 * (k + 1)],
                    hT_all[:, NTOK * k:NTOK * (k + 1)],
                    start=(k == 0), stop=(k == 7))
            ev = ev_pool.tile([128, NTOK], bf16, tag=f"ev{v % 3}")
            if v % 2 == 0:
                nc.vector.tensor_copy(ev[:], pf[:])
            else:
                nc.scalar.copy(ev[:], pf[:])
            nc.sync.dma_start(out_d[v], ev[:])

        # PSUM column layout per step (gate-major): col = g*16 + d*8 + q*2+b
        # with g in {i:0, f:1, chat:2, o:3}. Every elementwise operand is a
        # flat contiguous slice, and the i/f/chat block [0,48) excludes the
        # o matmuls from the ACT dependency.
        def gcol(d, m):
            return (m // 4) * 16 + 8 * d + 2 * (m % 4)

        # Accumulate with start=False everywhere onto pre-zeroed banks: a
        # start=True (first_mm) clears has_written for the WHOLE bank, which
        # breaks interleaved accumulation groups; accumulating onto zeroed
        # values is order-independent and hw-bit-agnostic. Both window
        # buffers are zeroed upfront; each step re-zeroes its block of the
        # next window right after its H write (GPSIMD cannot touch PSUM).
        nc.vector.memset(gw_bufs[0][:], 0.0)
        nc.vector.memset(gw_bufs[1][:], 0.0)
        n_win = t_steps // WIN
        for w in range(n_win):
            gw = gw_bufs[w % 2]
            gw3 = gw[:].rearrange("p (si c) -> p si c", si=WIN)
            # x part for the whole window: out [128, (8 si, 2 b)]
            for d in range(2):
                for k in range(2):
                    rhs = xT[d][k][:, w * 2 * WIN:(w + 1) * 2 * WIN] \
                        .rearrange("p (si b) -> p si b", si=WIN)
                    for m in range(16):
                        c0 = gcol(d, m)
                        nc.tensor.matmul(
                            gw3[:, :, c0:c0 + 2],
                            w_sb[d][k][:, 128 * m:128 * (m + 1)],
                            rhs, start=False, stop=False,
                            skip_group_check=True)
            for si in range(WIN):
                s = WIN * w + si
                # h part: i,f,chat tiles first; o tiles off the critical path
                for m0, m1 in ((0, 12), (12, 16)):
                    for d in range(2):
                        tok_prev = s - 1 if d == 0 else T - s
                        for m in range(m0, m1):
                            c0 = 64 * si + gcol(d, m)
                            for k in range(2, KR):
                                q = k - 2
                                if s == 0:
                                    rhs = hT_zero[:]
                                else:
                                    o = (4 * d + q) * NTOK + 2 * tok_prev
                                    rhs = hT_all[:, o:o + 2]
                                nc.tensor.matmul(
                                    gw[:, c0:c0 + 2],
                                    w_sb[d][k][:, 128 * m:128 * (m + 1)],
                                    rhs, start=False, stop=(k == KR - 1),
                                    skip_group_check=True)

                # ---- cell update ----
                gv = gw[:, 64 * si:64 * si + 64]
                if debug_dump:
                    dgt = ew_pool.tile([128, 64], f32, tag="dbgg")
                    nc.scalar.copy(dgt[:], gv)
                    nc.sync.dma_start(dbg_g[s], dgt[:])
                t_t = ew_pool.tile([128, 64], f32, tag="tt")
                # t = tanh(0.5*g): one op for i,f,chat; o separately (it is
                # only needed at the very end of the chain)
                nc.scalar.activation(t_t[:, 0:48], gv[:, 0:48],
                                     mybir.ActivationFunctionType.Tanh,
                                     scale=0.5)
                nc.scalar.activation(t_t[:, 48:64], gv[:, 48:64],
                                     mybir.ActivationFunctionType.Tanh,
                                     scale=0.5)
                qt = ew_pool.tile([128, 16], f32, tag="qt")
                nc.vector.scalar_tensor_tensor(
                    qt[:], t_t[:, 16:32], 1.0, c_sb[:],
                    op0=mybir.AluOpType.add, op1=mybir.AluOpType.mult)
                pt = ew_pool.tile([128, 16], f32, tag="pt")
                nc.vector.scalar_tensor_tensor(
                    pt[:], t_t[:, 0:16], 1.0, t_t[:, 32:48],
                    op0=mybir.AluOpType.add, op1=mybir.AluOpType.mult)
                # C' = 0.5*q + p  (C = 2c)
                nc.vector.scalar_tensor_tensor(
                    c_sb[:], qt[:], 0.5, pt[:],
                    op0=mybir.AluOpType.mult, op1=mybir.AluOpType.add)
                tanc = ew_pool.tile([128, 16], f32, tag="tanc")
                nc.scalar.activation(tanc[:], c_sb[:],
                                     mybir.ActivationFunctionType.Tanh,
                                     scale=0.5)
                # H = (t_o + 1) * tanh(c); straight into hT_all, bf16
                hT8 = hT_all[:].rearrange("p (kk n) -> p kk n", kk=8)
                for d in range(2):
                    tok = s if d == 0 else T - 1 - s
                    nc.vector.scalar_tensor_tensor(
                        hT8[:, 4 * d:4 * d + 4, 2 * tok:2 * tok + 2],
                        t_t[:, 48 + 8 * d:48 + 8 * d + 8]
                        .rearrange("p (q b) -> p q b", q=4),
                        1.0,
                        tanc[:, 8 * d:8 * d + 8]
                        .rearrange("p (q b) -> p q b", q=4),
                        op0=mybir.AluOpType.add, op1=mybir.AluOpType.mult)
                if w + 1 < n_win:
                    nc.vector.memset(
                        gw_bufs[(w + 1) % 2][:, 64 * si:64 * si + 64], 0.0)

                # FC interleave: middle cols of 2 vocab tiles per step once
                # tokens [64,192) are complete (s >= 192); fcw prefetched a
                # few steps ahead so FC matmuls never stall the PE queue.
                if t_steps == T:
                    sp = s + 3                       # prefetch lead
                    if T - NPRE // 2 <= sp < T:
                        for v in (2 * (sp - (T - NPRE // 2)),
                                  2 * (sp - (T - NPRE // 2)) + 1):
                            fc_stream(v)
                    if s >= T - NPRE // 2:
                        for v in (2 * (s - (T - NPRE // 2)),
                                  2 * (s - (T - NPRE // 2)) + 1):
                            fc_mid(v)

        if debug_dump:
            dh = ew_pool.tile([128, 8 * NTOK], f32, tag="dbgh")
            nc.vector.tensor_copy(dh[:], hT_all[:])
            nc.sync.dma_start(dbg_hT[:], dh[:])

        # ---- FC phase E: remaining full tiles, then edge cols of the
        # phase-R tiles (their fcw is re-streamed) ----
        if t_steps == T:
            order = list(range(NPRE, VT)) + list(range(NPRE))
            depth = RING - 2
            for v in order[:depth]:
                fc_stream(v, "A" if v >= NPRE else "E")
            for i, v in enumerate(order):
                if i + depth < len(order):
                    vn = order[i + depth]
                    fc_stream(vn, "A" if vn >= NPRE else "E")
                if v >= NPRE:
                    fc_full(v)
                else:
                    fc_edges(v)
        else:
            for v in range(VT):
                fc_stream(v)
                fc_full(v)

    nc.compile()
    return nc


def _host_prep(inputs, emb, Wh_fwd, Wx_fwd, b_fwd, Wh_bwd, Wx_bwd, b_bwd,
               fc_w, fc_b):
    inp = np.asarray(inputs).astype(np.int32)          # [B, T]
    emb = np.ascontiguousarray(np.asarray(emb, dtype=np.float32))

    wcat = np.zeros((2, KR, 128, 16 * 128), dtype=np.float32)
    for d, (Wh, Wx) in enumerate(((Wh_fwd, Wx_fwd), (Wh_bwd, Wx_bwd))):
        Wh = np.asarray(Wh, dtype=np.float32)
        Wx = np.asarray(Wx, dtype=np.float32)
        Wfull = np.zeros((E + H, 4 * H), dtype=np.float32)
        for gm in range(4):
            gr = GMAP[gm]
            Wfull[:E, gm * H:(gm + 1) * H] = Wx[gr]
            Wfull[E:, gm * H:(gm + 1) * H] = Wh[gr] * 0.5
        Wfull[:, 2 * H:3 * H] *= 2.0                   # chat columns
        wcat[d] = Wfull.reshape(KR, 128, 16 * 128)
    wcat = wcat.astype(ml_dtypes.bfloat16)

    fc_w = np.asarray(fc_w, dtype=np.float32) * 0.5    # H = 2h
    fcw = np.ascontiguousarray(
        fc_w.reshape(8, 128, VT, 128).transpose(2, 1, 0, 3)
        .reshape(VT, 128, 8 * 128)).astype(ml_dtypes.bfloat16)

    ident = np.eye(128, dtype=np.float32)
    ts = np.arange(T)
    in_maps = []
    for c in range(N_CORES):
        idx = np.zeros((8, 128), dtype=np.int32)
        for b in range(BL):
            loc = inp[BL * c + b]                      # [T]
            idx.reshape(2, 4 * 128)[0, 2 * ts + b] = loc
            idx.reshape(2, 4 * 128)[1, 2 * ts + b] = loc[::-1]
        in_maps.append(dict(idx=idx, emb=emb, wcat=wcat, fcw=fcw,
                            ident=ident))
    lstm_bias_zero = (not np.any(np.asarray(b_fwd))) and \
        (not np.any(np.asarray(b_bwd)))
    return in_maps, lstm_bias_zero


def run(in_maps, nc=None, **build_kw):
    if nc is None:
        key = tuple(sorted(build_kw.items()))
        if key not in _CACHE:
            _CACHE[key] = _build(**build_kw)
        nc = _CACHE[key]
    res = run_bass_kernel_spmd(nc, in_maps, core_ids=list(range(N_CORES)))
    return res


def kernel(**inputs):
    in_maps, lstm_bias_zero = _host_prep(**inputs)
    assert lstm_bias_zero, "nonzero LSTM biases not supported by this build"
    res = run(in_maps)
    ts = np.arange(T)
    out = np.empty((B, T, V), dtype=np.float32)
    for c in range(N_CORES):
        lg = np.asarray(res.results[c]["logits"]).reshape(V, NTOK)
        lg = lg.astype(np.float32)
        for b in range(BL):
            out[BL * c + b] = lg[:, 2 * ts + b].T
    fc_b = np.asarray(inputs["fc_b"], dtype=np.float32)
    if np.any(fc_b):
        out += fc_b
    return out
